# revision 14
# baseline (speedup 1.0000x reference)
"""MeshConvPoint Bass/Trainium2 kernel.

Problem (per mesh b of B=8, one NeuronCore each):
    nbr_mean[c,v] = (1/deg[v]) * sum_{d<deg[v]} x[c, nbr_idx[v,d]]
    out[o,v]     = sum_c W[o,c,0]*x[c,v] + W[o,c,1]*nbr_mean[c,v] + b[o]

Device strategy (vertex-major gather via SWDGE dma_gather):
  - x^T stored in DRAM as [NSRC, 64] f32 rows (256B each) with a zero row at
    index V; invalid neighbor slots and pad vertices point at the zero row.
  - Degree-sorted tiling: vertices sorted by degree on the host, grouped into
    128-vertex tiles; tile t has a static slot count s_t = max degree of the
    tile across all 8 cores, so the gather fetches ~mean-degree rows per
    vertex instead of D=12. The program is specialized to the slot profile.
  - Gather order j = (row_off_t + d)*128 + v_local lands a chunk as
    [128 parts = v_local, rows = (tile, slot), 64 ch].
  - Compute per chunk, instruction-count minimized:
      * one batched VectorE reduce + one broadcast multiply per equal-degree
        run of tiles (sum over slots, scaled by 1/deg)
      * per tile: TensorE transpose of the mean to channel-major, ScalarE
        copy into partitions 0..63 of a [128, cw] staging tile whose
        partitions 64..127 were DMA-filled with channel-major x (self term),
        then ONE matmul with Wcat = [W1T; W0T]
      * 4 tiles share one [64, 512] PSUM bank; one ScalarE bias-add per bank
  - Host un-permutes output columns.
"""

import numpy as np

import concourse.bacc as bacc
import concourse.mybir as mybir
from concourse import masks
from concourse.tile import TileContext
from concourse.bass_utils import run_bass_kernel_spmd

B, C, V, D, O = 8, 64, 25000, 12, 64

# per-dma_gather limits: 112*128=14336 indices stays under the ~16K-descriptor
# SWDGE carveout (21504 kills the device); 28 tiles bounds SBUF staging
MAX_CHUNK_ROWS = 112
MAX_CHUNK_TILES = 28


def _plan(v):
    nt = -(-v // 128)  # vertex tiles of 128
    return {
        "V": v,
        "NT": nt,
        "VP": nt * 128,
        "NSRC": ((v + 32) + 31) // 32 * 32,  # zero row at index v
    }


def _chunks_from_slots(slots):
    """Greedily pack tiles into gather chunks (row and tile caps).

    Returns a list of (tile_ids, row_offsets) per chunk. The final chunk is
    tapered into pieces of <= 3 tiles so the pipeline tail (compute after the
    last gather) stays short."""
    chunks = []
    cur, offs, rows = [], [], 0
    for t, s in enumerate(slots):
        if cur and (rows + s > MAX_CHUNK_ROWS or len(cur) >= MAX_CHUNK_TILES):
            chunks.append((cur, offs))
            cur, offs, rows = [], [], 0
        cur.append(t)
        offs.append(rows)
        rows += s
    if cur:
        chunks.append((cur, offs))
    if chunks:
        tail_ids, _ = chunks.pop()
        for i in range(0, len(tail_ids), 3):
            ids = tail_ids[i : i + 3]
            offs, r = [], 0
            for t in ids:
                offs.append(r)
                r += slots[t]
            chunks.append((ids, offs))
    return chunks


def _runs(tile_ids, row_offs, slots):
    """Group chunk-local tiles into runs of equal slot count.

    Yields (i0, n, s, r0): chunk-local start tile, run length, slots, row."""
    i = 0
    while i < len(tile_ids):
        s = slots[tile_ids[i]]
        j = i
        while j < len(tile_ids) and slots[tile_ids[j]] == s:
            j += 1
        yield i, j - i, s, row_offs[i]
        i = j


def build_nc(p, slots):
    f32 = mybir.dt.float32
    chunks = _chunks_from_slots(slots)
    total_idx = 128 * sum(slots)
    idx_cols_total = total_idx // 16

    nc = bacc.Bacc()
    xT = nc.declare_dram_parameter("xT", [p["NSRC"], C], f32, isOutput=False)
    xc_d = nc.declare_dram_parameter("xc", [C, p["VP"]], f32, isOutput=False)
    idx16 = nc.declare_dram_parameter(
        "idx16", [128, idx_cols_total], mybir.dt.int16, isOutput=False
    )
    invdeg = nc.declare_dram_parameter("invdeg", [128, p["NT"]], f32, isOutput=False)
    wcat_d = nc.declare_dram_parameter("wcat", [2 * C, O], f32, isOutput=False)
    bias = nc.declare_dram_parameter("bias", [O, 1], f32, isOutput=False)
    out = nc.declare_dram_parameter("out", [O, p["VP"]], f32, isOutput=True)

    with TileContext(nc) as tc:
        with (
            tc.tile_pool(name="const", bufs=1) as cpool,
            tc.tile_pool(name="idxp", bufs=3) as idxpool,
            tc.tile_pool(name="gp", bufs=2) as gpool,
            tc.tile_pool(name="xcp", bufs=2) as xcpool,
            tc.tile_pool(name="stp", bufs=2) as stpool,
            tc.tile_pool(name="outp", bufs=2) as outpool,
            tc.tile_pool(name="psgp", bufs=4, space="PSUM") as psgpool,
            tc.tile_pool(name="psop", bufs=3, space="PSUM") as psopool,
        ):
            invd = cpool.tile([128, p["NT"]], f32)
            nc.sync.dma_start(out=invd[:, :], in_=invdeg[:, :])
            wcat = cpool.tile([2 * C, O], f32)
            nc.sync.dma_start(out=wcat[:, :], in_=wcat_d[:, :])
            bb = cpool.tile([O, 1], f32)
            nc.sync.dma_start(out=bb[:, :], in_=bias[:, :])
            ident = cpool.tile([128, 128], f32)
            masks.make_identity(nc, ident[:, :])

            idx_off = 0  # running idx column offset into idx16
            for tile_ids, row_offs in chunks:
                ntl = len(tile_ids)
                crows = row_offs[-1] + slots[tile_ids[-1]]
                cidx = crows * 128
                icols = cidx // 16
                cw = ntl * 128
                c0 = tile_ids[0] * 128  # first output column of this chunk

                idxb = idxpool.tile([128, icols], mybir.dt.int16, tag="idxb")
                # ACT HWDGE queue: don't serialize behind the const DMAs on SP
                nc.scalar.dma_start(
                    out=idxb[:, :], in_=idx16[:, idx_off : idx_off + icols]
                )
                idx_off += icols
                g = gpool.tile([128, crows, C], f32, tag="g")
                nc.gpsimd.dma_gather(
                    g[:, :, :],
                    xT[:, :],
                    idxb[:, :],
                    cidx,
                    cidx,
                    C,
                    # one packet per instruction deadlocks the SWDGE ring once
                    # descriptors exceed the carveout
                    single_packet=False,
                )
                # staging: partitions 64..127 = channel-major x (self term),
                # partitions 0..63 get the transposed neighbor means per tile
                xcb = xcpool.tile([128, cw], f32, tag="xcb")
                nc.scalar.dma_start(out=xcb[64:128, :], in_=xc_d[:, c0 : c0 + cw])
                # batched neighbor mean per equal-degree run
                stb = stpool.tile([128, ntl * C], f32, tag="stb")
                for i0, n, s, r0 in _runs(tile_ids, row_offs, slots):
                    t0 = tile_ids[i0]
                    dst = stb[:, i0 * C : (i0 + n) * C].rearrange(
                        "p (n c) -> p n c", c=C
                    )
                    nc.vector.reduce_sum(
                        out=dst,
                        in_=g[:, r0 : r0 + n * s, :].rearrange(
                            "p (n s) c -> p n c s", s=s
                        ),
                        axis=mybir.AxisListType.X,
                    )
                    nc.vector.tensor_mul(
                        dst,
                        dst,
                        invd[:, t0 : t0 + n].unsqueeze(2).broadcast_to([128, n, C]),
                    )
                outst = outpool.tile([O, cw], f32, tag="outst")
                pso = None
                for i in range(ntl):
                    psg = psgpool.tile([O, 128], f32, tag="psg")
                    nc.tensor.transpose(
                        psg[:, :], stb[:, i * C : (i + 1) * C], ident[:, :]
                    )
                    nc.scalar.copy(xcb[0:64, i * 128 : (i + 1) * 128], psg[:, :])
                    if i % 4 == 0:
                        pso = psopool.tile([O, 512], f32, tag="pso")
                    nc.tensor.matmul(
                        pso[:, (i % 4) * 128 : (i % 4 + 1) * 128],
                        lhsT=wcat[:, :],
                        rhs=xcb[:, i * 128 : (i + 1) * 128],
                        start=True,
                        stop=True,
                    )
                    if i % 4 == 3 or i == ntl - 1:
                        k = i % 4 + 1
                        nc.scalar.add(
                            outst[:, (i - k + 1) * 128 : (i + 1) * 128],
                            pso[:, : k * 128],
                            add=bb[:, 0:1],
                        )
                nc.sync.dma_start(out=out[:, c0 : c0 + cw], in_=outst[:, :])
    nc.finalize()
    return nc


def degree_sort(deg_all, p):
    """Shared tiling across cores: per-core ascending-degree vertex order and
    the per-tile static slot counts (max degree in the tile over all cores)."""
    v, vp, nt = p["V"], p["VP"], p["NT"]
    nb = deg_all.shape[0]
    orders = []
    degs_sorted = []
    for bi in range(nb):
        dfull = np.zeros(vp, np.int64)
        dfull[:v] = deg_all[bi]
        order = np.argsort(dfull, kind="stable")
        orders.append(order)
        degs_sorted.append(dfull[order])
    degs_sorted = np.stack(degs_sorted)  # [nb, vp]
    tile_max = degs_sorted.reshape(nb, nt, 128).max(axis=(0, 2))
    slots = np.maximum(tile_max, 1).astype(int).tolist()
    return orders, slots


def host_prep(x, nbr_idx, deg, W, b, p, orders, slots):
    """Per-core input maps: layout/sharding prep only (no math on x)."""
    v, vp, nt, nsrc = p["V"], p["VP"], p["NT"], p["NSRC"]
    # Wcat rows 0..63 multiply the neighbor mean (partitions 0..63 of the
    # staging tile), rows 64..127 the self features
    wcat = np.concatenate([W[:, :, 1].T, W[:, :, 0].T], axis=0).astype(np.float32)
    bvec = np.ascontiguousarray(b.reshape(O, 1), dtype=np.float32)
    nb = x.shape[0]
    in_maps = []
    for bi in range(nb):
        order = orders[bi]
        valid = order < v
        xT = np.zeros((nsrc, C), np.float32)
        xT[:v] = x[bi].T
        xc = np.zeros((C, vp), np.float32)
        xc[:, valid] = x[bi][:, order[valid]]
        dfull = np.zeros(vp, np.int64)
        dfull[:v] = deg[bi]
        deg_s = dfull[order]  # [vp]
        # neighbor table in sorted order, padded to the static slot profile
        nbr_s = np.full((vp, D), v, np.int32)
        nbr_s[valid] = np.where(
            np.arange(D)[None, :] < deg_s[valid][:, None],
            nbr_idx[bi][order[valid]],
            v,
        )
        # gather index stream: per tile t, slots[t] rows of 128 lanes
        parts = []
        nbr_tiles = nbr_s.reshape(nt, 128, D)
        for t in range(nt):
            parts.append(nbr_tiles[t, :, : slots[t]].T)  # [s_t, 128]
        arr = np.concatenate(parts, axis=0).reshape(-1)
        idx16 = np.tile(
            np.ascontiguousarray(arr.reshape(-1, 16).T).astype(np.int16), (8, 1)
        )
        invdeg = np.ascontiguousarray(
            (1.0 / np.maximum(deg_s, 1).astype(np.float32)).reshape(nt, 128).T
        )
        in_maps.append(
            {
                "xT": xT,
                "xc": xc,
                "idx16": np.ascontiguousarray(idx16),
                "invdeg": invdeg,
                "wcat": wcat,
                "bias": bvec,
            }
        )
    return in_maps


_CACHE = {}
TRACE = False
LAST_RESULT = None


def _get_nc(p, slots):
    key = (p["V"], tuple(slots))
    if key not in _CACHE:
        _CACHE[key] = build_nc(p, slots)
    return _CACHE[key]


def kernel(x, nbr_idx, deg, W, b):
    global LAST_RESULT
    x = np.asarray(x, np.float32)
    nbr_idx = np.asarray(nbr_idx, np.int32)
    deg = np.asarray(deg, np.int32)
    W = np.asarray(W, np.float32)
    b = np.asarray(b, np.float32)
    p = _plan(x.shape[2])
    orders, slots = degree_sort(deg, p)
    in_maps = host_prep(x, nbr_idx, deg, W, b, p, orders, slots)
    nc = _get_nc(p, slots)
    try:
        res = run_bass_kernel_spmd(nc, in_maps, list(range(len(in_maps))), trace=TRACE)
    except ModuleNotFoundError:
        res = run_bass_kernel_spmd(nc, in_maps, list(range(len(in_maps))), trace=False)
    LAST_RESULT = res
    v = p["V"]
    outs = []
    for bi, r in enumerate(res.results):
        order = orders[bi]
        valid = order < v
        ob = np.empty((O, v), np.float32)
        ob[:, order[valid]] = r["out"][:, valid]
        outs.append(ob)
    out = np.stack(outs, axis=0)
    return out[..., None].astype(np.float32)


# revision 16
# speedup vs baseline: 1.0999x; 1.0999x over previous
"""MeshConvPoint Bass/Trainium2 kernel.

Problem (per mesh b of B=8, one NeuronCore each):
    nbr_mean[c,v] = (1/deg[v]) * sum_{d<deg[v]} x[c, nbr_idx[v,d]]
    out[o,v]     = sum_c W[o,c,0]*x[c,v] + W[o,c,1]*nbr_mean[c,v] + b[o]

Device strategy (vertex-major gather via SWDGE dma_gather):
  - x^T stored in DRAM as [NSRC, 64] f32 rows (256B each) with a zero row at
    index V; invalid neighbor slots and pad vertices point at the zero row.
  - Degree-sorted tiling: vertices sorted by degree on the host, grouped into
    128-vertex tiles; tile t has a static slot count s_t = max degree of the
    tile across all 8 cores, so the gather fetches ~mean-degree rows per
    vertex instead of D=12. The program is specialized to the slot profile.
  - Gather order j = (row_off_t + d)*128 + v_local lands a chunk as
    [128 parts = v_local, rows = (tile, slot), 64 ch].
  - Compute per chunk, instruction-count minimized:
      * one batched VectorE reduce + one broadcast multiply per equal-degree
        run of tiles (sum over slots, scaled by 1/deg)
      * per tile: TensorE transpose of the mean to channel-major, ScalarE
        copy into partitions 0..63 of a [128, cw] staging tile whose
        partitions 64..127 were DMA-filled with channel-major x (self term),
        then ONE matmul with Wcat = [W1T; W0T]
      * 4 tiles share one [64, 512] PSUM bank; one ScalarE bias-add per bank
  - Host un-permutes output columns.
"""

import numpy as np

import concourse.bacc as bacc
import concourse.mybir as mybir
from concourse import masks
from concourse.tile import TileContext
from concourse.bass_utils import run_bass_kernel_spmd

B, C, V, D, O = 8, 64, 25000, 12, 64

# per-dma_gather limits: 112*128=14336 indices stays under the ~16K-descriptor
# SWDGE carveout (21504 kills the device); 28 tiles bounds SBUF staging
MAX_CHUNK_ROWS = 112
MAX_CHUNK_TILES = 28


def _plan(v):
    nt = -(-v // 128)  # vertex tiles of 128
    return {
        "V": v,
        "NT": nt,
        "VP": nt * 128,
        "NSRC": ((v + 32) + 31) // 32 * 32,  # zero row at index v
    }


def _chunks_from_slots(slots):
    """Greedily pack tiles into gather chunks (row and tile caps).

    Returns a list of (tile_ids, row_offsets) per chunk. The final chunk is
    tapered into pieces of <= 3 tiles so the pipeline tail (compute after the
    last gather) stays short."""
    chunks = []
    cur, offs, rows = [], [], 0
    for t, s in enumerate(slots):
        if cur and (rows + s > MAX_CHUNK_ROWS or len(cur) >= MAX_CHUNK_TILES):
            chunks.append((cur, offs))
            cur, offs, rows = [], [], 0
        cur.append(t)
        offs.append(rows)
        rows += s
    if cur:
        chunks.append((cur, offs))
    if chunks:
        tail_ids, _ = chunks.pop()
        for i in range(0, len(tail_ids), 3):
            ids = tail_ids[i : i + 3]
            offs, r = [], 0
            for t in ids:
                offs.append(r)
                r += slots[t]
            chunks.append((ids, offs))
    return chunks


def _runs(tile_ids, row_offs, slots):
    """Group chunk-local tiles into runs of equal slot count.

    Yields (i0, n, s, r0): chunk-local start tile, run length, slots, row."""
    i = 0
    while i < len(tile_ids):
        s = slots[tile_ids[i]]
        j = i
        while j < len(tile_ids) and slots[tile_ids[j]] == s:
            j += 1
        yield i, j - i, s, row_offs[i]
        i = j


def build_nc(p, slots):
    f32 = mybir.dt.float32
    chunks = _chunks_from_slots(slots)
    total_idx = 128 * sum(slots)
    idx_cols_total = total_idx // 16

    nc = bacc.Bacc()
    xT = nc.declare_dram_parameter("xT", [p["NSRC"], C], f32, isOutput=False)
    xc_d = nc.declare_dram_parameter("xc", [C, p["VP"]], f32, isOutput=False)
    idx16 = nc.declare_dram_parameter(
        "idx16", [128, idx_cols_total], mybir.dt.int16, isOutput=False
    )
    invdeg = nc.declare_dram_parameter("invdeg", [128, p["NT"]], f32, isOutput=False)
    wcat_d = nc.declare_dram_parameter("wcat", [2 * C, O], f32, isOutput=False)
    bias = nc.declare_dram_parameter("bias", [O, 1], f32, isOutput=False)
    out = nc.declare_dram_parameter("out", [O, p["VP"]], f32, isOutput=True)

    with TileContext(nc) as tc:
        with (
            tc.tile_pool(name="const", bufs=1) as cpool,
            tc.tile_pool(name="idxp", bufs=3) as idxpool,
            tc.tile_pool(name="gp", bufs=2) as gpool,
            tc.tile_pool(name="xcp", bufs=2) as xcpool,
            tc.tile_pool(name="stp", bufs=2) as stpool,
            tc.tile_pool(name="outp", bufs=2) as outpool,
            tc.tile_pool(name="psgp", bufs=4, space="PSUM") as psgpool,
            tc.tile_pool(name="psop", bufs=3, space="PSUM") as psopool,
        ):
            invd = cpool.tile([128, p["NT"]], f32)
            nc.sync.dma_start(out=invd[:, :], in_=invdeg[:, :])
            wcat = cpool.tile([2 * C, O], f32)
            nc.sync.dma_start(out=wcat[:, :], in_=wcat_d[:, :])
            bb = cpool.tile([O, 1], f32)
            nc.sync.dma_start(out=bb[:, :], in_=bias[:, :])
            ident = cpool.tile([128, 128], f32)
            masks.make_identity(nc, ident[:, :])

            idx_off = 0  # running idx column offset into idx16
            for tile_ids, row_offs in chunks:
                ntl = len(tile_ids)
                crows = row_offs[-1] + slots[tile_ids[-1]]
                cidx = crows * 128
                icols = cidx // 16
                cw = ntl * 128
                c0 = tile_ids[0] * 128  # first output column of this chunk

                idxb = idxpool.tile([128, icols], mybir.dt.int16, tag="idxb")
                nc.sync.dma_start(
                    out=idxb[:, :], in_=idx16[:, idx_off : idx_off + icols]
                )
                idx_off += icols
                g = gpool.tile([128, crows, C], f32, tag="g")
                nc.gpsimd.dma_gather(
                    g[:, :, :],
                    xT[:, :],
                    idxb[:, :],
                    cidx,
                    cidx,
                    C,
                    # one packet per instruction deadlocks the SWDGE ring once
                    # descriptors exceed the carveout
                    single_packet=False,
                )
                # staging: partitions 64..127 = channel-major x (self term),
                # partitions 0..63 get the transposed neighbor means per tile
                xcb = xcpool.tile([128, cw], f32, tag="xcb")
                nc.sync.dma_start(out=xcb[64:128, :], in_=xc_d[:, c0 : c0 + cw])
                # batched neighbor mean per equal-degree run
                stb = stpool.tile([128, ntl * C], f32, tag="stb")
                for i0, n, s, r0 in _runs(tile_ids, row_offs, slots):
                    t0 = tile_ids[i0]
                    dst = stb[:, i0 * C : (i0 + n) * C].rearrange(
                        "p (n c) -> p n c", c=C
                    )
                    nc.vector.reduce_sum(
                        out=dst,
                        in_=g[:, r0 : r0 + n * s, :].rearrange(
                            "p (n s) c -> p n c s", s=s
                        ),
                        axis=mybir.AxisListType.X,
                    )
                    nc.vector.tensor_mul(
                        dst,
                        dst,
                        invd[:, t0 : t0 + n].unsqueeze(2).broadcast_to([128, n, C]),
                    )
                outst = outpool.tile([O, cw], f32, tag="outst")
                pso = None
                for i in range(ntl):
                    psg = psgpool.tile([O, 128], f32, tag="psg")
                    nc.tensor.transpose(
                        psg[:, :], stb[:, i * C : (i + 1) * C], ident[:, :]
                    )
                    nc.scalar.copy(xcb[0:64, i * 128 : (i + 1) * 128], psg[:, :])
                    if i % 4 == 0:
                        pso = psopool.tile([O, 512], f32, tag="pso")
                    nc.tensor.matmul(
                        pso[:, (i % 4) * 128 : (i % 4 + 1) * 128],
                        lhsT=wcat[:, :],
                        rhs=xcb[:, i * 128 : (i + 1) * 128],
                        start=True,
                        stop=True,
                    )
                    if i % 4 == 3 or i == ntl - 1:
                        k = i % 4 + 1
                        nc.scalar.add(
                            outst[:, (i - k + 1) * 128 : (i + 1) * 128],
                            pso[:, : k * 128],
                            add=bb[:, 0:1],
                        )
                nc.sync.dma_start(out=out[:, c0 : c0 + cw], in_=outst[:, :])
    nc.finalize()
    return nc


def degree_sort(deg_all, p):
    """Shared tiling across cores: per-core ascending-degree vertex order and
    the per-tile static slot counts (max degree in the tile over all cores)."""
    v, vp, nt = p["V"], p["VP"], p["NT"]
    nb = deg_all.shape[0]
    orders = []
    degs_sorted = []
    for bi in range(nb):
        dfull = np.zeros(vp, np.int64)
        dfull[:v] = deg_all[bi]
        order = np.argsort(dfull, kind="stable")
        orders.append(order)
        degs_sorted.append(dfull[order])
    degs_sorted = np.stack(degs_sorted)  # [nb, vp]
    tile_max = degs_sorted.reshape(nb, nt, 128).max(axis=(0, 2))
    slots = np.maximum(tile_max, 1).astype(int).tolist()
    return orders, slots


def host_prep(x, nbr_idx, deg, W, b, p, orders, slots):
    """Per-core input maps: layout/sharding prep only (no math on x)."""
    v, vp, nt, nsrc = p["V"], p["VP"], p["NT"], p["NSRC"]
    # Wcat rows 0..63 multiply the neighbor mean (partitions 0..63 of the
    # staging tile), rows 64..127 the self features
    wcat = np.concatenate([W[:, :, 1].T, W[:, :, 0].T], axis=0).astype(np.float32)
    bvec = np.ascontiguousarray(b.reshape(O, 1), dtype=np.float32)
    nb = x.shape[0]
    in_maps = []
    for bi in range(nb):
        order = orders[bi]
        valid = order < v
        xT = np.zeros((nsrc, C), np.float32)
        xT[:v] = x[bi].T
        xc = np.zeros((C, vp), np.float32)
        xc[:, valid] = x[bi][:, order[valid]]
        dfull = np.zeros(vp, np.int64)
        dfull[:v] = deg[bi]
        deg_s = dfull[order]  # [vp]
        # neighbor table in sorted order, padded to the static slot profile
        nbr_s = np.full((vp, D), v, np.int32)
        nbr_s[valid] = np.where(
            np.arange(D)[None, :] < deg_s[valid][:, None],
            nbr_idx[bi][order[valid]],
            v,
        )
        # gather index stream: per tile t, slots[t] rows of 128 lanes
        parts = []
        nbr_tiles = nbr_s.reshape(nt, 128, D)
        for t in range(nt):
            parts.append(nbr_tiles[t, :, : slots[t]].T)  # [s_t, 128]
        arr = np.concatenate(parts, axis=0).reshape(-1)
        idx16 = np.tile(
            np.ascontiguousarray(arr.reshape(-1, 16).T).astype(np.int16), (8, 1)
        )
        invdeg = np.ascontiguousarray(
            (1.0 / np.maximum(deg_s, 1).astype(np.float32)).reshape(nt, 128).T
        )
        in_maps.append(
            {
                "xT": xT,
                "xc": xc,
                "idx16": np.ascontiguousarray(idx16),
                "invdeg": invdeg,
                "wcat": wcat,
                "bias": bvec,
            }
        )
    return in_maps


_CACHE = {}
TRACE = False
LAST_RESULT = None


def _get_nc(p, slots):
    key = (p["V"], tuple(slots))
    if key not in _CACHE:
        _CACHE[key] = build_nc(p, slots)
    return _CACHE[key]


def kernel(x, nbr_idx, deg, W, b):
    global LAST_RESULT
    x = np.asarray(x, np.float32)
    nbr_idx = np.asarray(nbr_idx, np.int32)
    deg = np.asarray(deg, np.int32)
    W = np.asarray(W, np.float32)
    b = np.asarray(b, np.float32)
    p = _plan(x.shape[2])
    orders, slots = degree_sort(deg, p)
    in_maps = host_prep(x, nbr_idx, deg, W, b, p, orders, slots)
    nc = _get_nc(p, slots)
    try:
        res = run_bass_kernel_spmd(nc, in_maps, list(range(len(in_maps))), trace=TRACE)
    except ModuleNotFoundError:
        res = run_bass_kernel_spmd(nc, in_maps, list(range(len(in_maps))), trace=False)
    LAST_RESULT = res
    v = p["V"]
    outs = []
    for bi, r in enumerate(res.results):
        order = orders[bi]
        valid = order < v
        ob = np.empty((O, v), np.float32)
        ob[:, order[valid]] = r["out"][:, valid]
        outs.append(ob)
    out = np.stack(outs, axis=0)
    return out[..., None].astype(np.float32)
